# revision 47
# baseline (speedup 1.0000x reference)
"""Trainium2 Bass kernel for nn_Decoder: 2-layer LSTM decoder with autoregressive
feedback (B=1024, T=256, H=1024), data-parallel over 8 NeuronCores.

Strategy:
  - Shard batch 1024 -> 128 per core; replicate weights (streamed/resident).
  - Per-core layout: batch on partitions, gate features on free dim.
  - fp16 matmul operands (weights + h), fp32 PSUM accumulation, fp32 cell state.
  - Gate rows pre-permuted on host into 8 chunks of [i|f|o|g] x 128 so one PSUM
    bank holds a complete gate quad for a 128-wide hidden slice.
  - bias + x-feedback folded into matmul via a K=2 stationary [ones; x].
  - h transposed each step on the PE (identity matmul) into fp16 h^T chunks.
  - W_hh0, W_hh1 resident in SBUF; W_ih1 streamed per step (gate-chunk blocks).
"""

import os

import numpy as np

import concourse.bass as bass
import concourse.mybir as mybir
import concourse.tile as tile
from concourse import bacc
from concourse.bass import ds
from concourse.bass_utils import run_bass_kernel_spmd

MM_DTYPE = os.environ.get("MM_DTYPE", "f16")
F16 = mybir.dt.float16 if MM_DTYPE == "f16" else mybir.dt.bfloat16
F32 = mybir.dt.float32
AF = mybir.ActivationFunctionType

DBG_NO_BIAS_MM = bool(int(os.environ.get("DBG_NO_BIAS_MM", "0")))
DBG_NO_TRANSPOSE = bool(int(os.environ.get("DBG_NO_TRANSPOSE", "0")))
DBG_NO_PRED = bool(int(os.environ.get("DBG_NO_PRED", "0")))
DBG_NO_STREAM = bool(int(os.environ.get("DBG_NO_STREAM", "0")))  # timing only
DBG_MM_ONLY = bool(int(os.environ.get("DBG_MM_ONLY", "0")))      # timing only

H = 1024
FOURH = 4 * H
KC = 8          # contraction chunks (H / 128)
NCH = 8         # gate chunks per layer (4096 / 512)
B_LOC = 128
N_CORES = 8
T_TOTAL = 256


def _gate_perm():
    """Row permutation of the 4H gate dim: chunk n = [i_n | f_n | o_n | g_n],
    each 128 wide, covering hidden slice [128n, 128(n+1))."""
    idx = np.arange(128)
    parts = []
    for n in range(NCH):
        base = n * 128
        parts += [base + idx, H + base + idx, 3 * H + base + idx, 2 * H + base + idx]
    return np.concatenate(parts)


def _build_nc(T, unroll, passes=1):
    """Build the Bass module (single-core program, replicated SPMD on 8 cores).

    passes > 1 repeats the whole T-step loop (timing builds only: wall-clock
    difference between passes isolates pure device time)."""
    assert T % unroll == 0 and unroll % 2 == 0
    groups = (T // unroll) * passes

    nc = bacc.Bacc("TRN2", target_bir_lowering=False, debug=False)

    dI = lambda name, shape, dt: nc.dram_tensor(name, shape, dt, kind="ExternalInput")
    w0_d = dI("w0", [128, KC * FOURH], F16)       # W_hh0^T, k-chunk blocks
    w1h_d = dI("w1h", [128, KC * FOURH], F16)     # W_hh1^T, k-chunk blocks
    w1i_d = dI("w1i", [128, NCH * FOURH], F16)    # W_ih1^T, gate-chunk blocks
    # K=32-padded bias/x operands (K<32 matmuls hang TRN2):
    #   wxb0: row0 = W_ih0 (perm), row1 = b_ih0+b_hh0 (perm), rows 2-31 = 0
    #   b1k:  row0 = b_ih1+b_hh1 (perm), rows 1-31 = 0
    wxb0_d = dI("wxb0", [32, FOURH], F16)
    b1k_d = dI("b1k", [32, FOURH], F16)
    # xk32: row0 = x (runtime), row1 = ones, rest 0; ones32: row0 = ones
    xk0_d = dI("xk0", [32, 128], F16)
    ones32_d = dI("ones32", [32, 128], F16)
    hT0_d = dI("hT0", [128, H], F16)
    hT1_d = dI("hT1", [128, H], F16)
    c0_d = dI("c0s", [128, H], F32)
    c1_d = dI("c1s", [128, H], F32)
    wout_d = dI("woutT", [128, KC], F16)
    boutb_d = dI("boutb", [128, 1], F32)
    bouts_d = dI("bouts", [1, 1], F32)
    ident_d = dI("ident", [128, 128], F16)
    preds_d = nc.dram_tensor("preds", [128, T * passes], F32,
                             kind="ExternalOutput")

    with tile.TileContext(nc) as tc:
        with (
            tc.tile_pool(name="state", bufs=1) as state,
            tc.tile_pool(name="wstream", bufs=2) as wstream,
            tc.tile_pool(name="work", bufs=3) as work,
            tc.tile_pool(name="gpsum", bufs=3, space="PSUM") as gpsum,
            tc.tile_pool(name="tpsum", bufs=3, space="PSUM") as tpsum,
            tc.tile_pool(name="ppsum", bufs=1, space="PSUM") as ppsum,
        ):
            # ---- persistent SBUF state -------------------------------------
            w0_sb = state.tile([128, KC * FOURH], F16, name="w0_sb")
            w1h_sb = state.tile([128, KC * FOURH], F16, name="w1h_sb")
            wxb0_sb = state.tile([32, FOURH], F16, name="wxb0_sb")
            b1k_sb = state.tile([32, FOURH], F16, name="b1k_sb")
            hT0_pp = [state.tile([128, H], F16, name=f"hT0_{p}") for p in range(2)]
            hT1_pp = [state.tile([128, H], F16, name=f"hT1_{p}") for p in range(2)]
            h0st = state.tile([128, H], F16, name="h0st")
            h1st = state.tile([128, H], F16, name="h1st")
            c0_sb = state.tile([128, H], F32, name="c0_sb")
            c1_sb = state.tile([128, H], F32, name="c1_sb")
            xk32 = state.tile([32, 128], F16, name="xk32")
            ones32 = state.tile([32, 128], F16, name="ones32")
            ident = state.tile([128, 128], F16, name="ident")
            woutT = state.tile([128, KC], F16, name="woutT")
            boutb = state.tile([128, 1], F32, name="boutb")
            bouts = state.tile([1, 1], F32, name="bouts")
            pring = state.tile([128, unroll], F32, name="pring")

            # ---- setup DMAs ------------------------------------------------
            nc.sync.dma_start(w0_sb, w0_d[:, :])
            nc.sync.dma_start(w1h_sb, w1h_d[:, :])
            nc.sync.dma_start(wxb0_sb, wxb0_d[:, :])
            nc.sync.dma_start(b1k_sb, b1k_d[:, :])
            nc.sync.dma_start(hT0_pp[0], hT0_d[:, :])
            nc.sync.dma_start(hT1_pp[0], hT1_d[:, :])
            nc.sync.dma_start(c0_sb, c0_d[:, :])
            nc.sync.dma_start(c1_sb, c1_d[:, :])
            nc.sync.dma_start(xk32, xk0_d[:, :])
            nc.sync.dma_start(ones32, ones32_d[:, :])
            nc.sync.dma_start(ident, ident_d[:, :])

            if DBG_NO_STREAM:
                w1i_fake = state.tile([128, FOURH], F16, name="w1i_fake")
                nc.sync.dma_start(w1i_fake, w1i_d[:, 0:FOURH])
            nc.sync.dma_start(woutT, wout_d[:, :])
            nc.sync.dma_start(boutb, boutb_d[:, :])
            nc.sync.dma_start(bouts, bouts_d[:, :])

            def lstm_layer(n, psum_mms, c_sb, hst, hT_new, bias_mms):
                """Emit gate-chunk n: matmuls into PSUM, activations, cell
                update, h write (fp16) and PE transpose into hT_new."""
                nsl = slice(n * 128, (n + 1) * 128)
                ps = gpsum.tile([128, 512], F32, tag="g", name="ps")
                if DBG_NO_BIAS_MM:
                    bias_mms = []
                psum_mms(ps, last=not bias_mms)
                for bi, (b_lhsT, b_rhs) in enumerate(bias_mms):
                    nc.tensor.matmul(ps, b_lhsT, b_rhs, start=False,
                                     stop=(bi == len(bias_mms) - 1))
                if DBG_MM_ONLY:
                    return
                sif = work.tile([128, 384], F32, tag="sif", name="sif")
                nc.scalar.activation(sif, ps[:, 0:384], AF.Sigmoid)
                tg = work.tile([128, 128], F32, tag="tg", name="tg")
                nc.scalar.activation(tg, ps[:, 384:512], AF.Tanh)
                prod = work.tile([128, 128], F32, tag="prod", name="prod")
                nc.vector.tensor_mul(prod, sif[:, 0:128], tg)
                tmp = work.tile([128, 128], F32, tag="tmp", name="tmp")
                nc.vector.tensor_mul(tmp, sif[:, 128:256], c_sb[:, nsl])
                nc.vector.tensor_add(c_sb[:, nsl], tmp, prod)
                tct = work.tile([128, 128], F32, tag="tct", name="tct")
                nc.scalar.activation(tct, c_sb[:, nsl], AF.Tanh)
                nc.vector.tensor_mul(hst[:, nsl], sif[:, 256:384], tct)
                if DBG_NO_TRANSPOSE:
                    nc.vector.tensor_copy(hT_new[:, nsl], hst[:, nsl])
                else:
                    pt = tpsum.tile([128, 128], F16, tag="tp", name="pt")
                    nc.tensor.transpose(pt, hst[:, nsl], ident)
                    nc.vector.tensor_copy(hT_new[:, nsl], pt)

            def emit_step(u):
                hT0_cur, hT0_new = hT0_pp[u % 2], hT0_pp[(u + 1) % 2]
                hT1_cur, hT1_new = hT1_pp[u % 2], hT1_pp[(u + 1) % 2]
                if DBG_MM_ONLY:
                    hT0_cur = hT0_new = hT0_pp[0]
                    hT1_cur = hT1_new = hT1_pp[0]

                # stream W_ih1 gate-chunk blocks for this step
                if DBG_NO_STREAM:
                    st = [w1i_fake] * NCH
                else:
                    st = []
                    for n in range(NCH):
                        s = wstream.tile([128, FOURH], F16, tag="w1is", name="w1is")
                        nc.sync.dma_start(s, w1i_d[:, n * FOURH:(n + 1) * FOURH])
                        st.append(s)

                # ---- layer 0 ----
                for n in range(NCH):
                    def mms0(ps, last=False, n=n):
                        for k in range(KC):
                            nc.tensor.matmul(
                                ps,
                                hT0_cur[:, k * 128:(k + 1) * 128],
                                w0_sb[:, k * FOURH + n * 512: k * FOURH + (n + 1) * 512],
                                start=(k == 0), stop=(last and k == KC - 1),
                            )
                    lstm_layer(n, mms0, c0_sb, h0st, hT0_new, [
                        (xk32[:, :], wxb0_sb[:, n * 512:(n + 1) * 512]),
                    ])

                # ---- layer 1 ----
                for n in range(NCH):
                    def mms1(ps, last=False, n=n):
                        for k in range(KC):
                            nc.tensor.matmul(
                                ps,
                                hT1_cur[:, k * 128:(k + 1) * 128],
                                w1h_sb[:, k * FOURH + n * 512: k * FOURH + (n + 1) * 512],
                                start=(k == 0), stop=False,
                            )
                        for k in range(KC):
                            nc.tensor.matmul(
                                ps,
                                hT0_new[:, k * 128:(k + 1) * 128],
                                st[n][:, k * 512:(k + 1) * 512],
                                start=False, stop=(last and k == KC - 1),
                            )
                    lstm_layer(n, mms1, c1_sb, h1st, hT1_new, [
                        (ones32[:, :], b1k_sb[:, n * 512:(n + 1) * 512]),
                    ])

                # ---- prediction ----
                if DBG_NO_PRED:
                    nc.scalar.activation(pring[:, u:u + 1], c1_sb[:, 0:1],
                                         AF.Identity, bias=boutb)
                    return
                # row form [1, 128] -> next-step x (fp16, into xones row 1)
                pr = ppsum.tile([1, 128], F32, tag="predr", name="pr")
                for k in range(KC):
                    nc.tensor.matmul(pr, woutT[:, k:k + 1],
                                     hT1_new[:, k * 128:(k + 1) * 128],
                                     start=(k == 0), stop=(k == KC - 1))
                nc.scalar.activation(xk32[0:1, :], pr, AF.Identity, bias=bouts)
                # column form [128, 1] -> preds ring (fp32)
                pc = ppsum.tile([128, 1], F32, tag="predc", name="pc")
                for k in range(KC):
                    nc.tensor.matmul(pc, hT1_new[:, k * 128:(k + 1) * 128],
                                     woutT[:, k:k + 1],
                                     start=(k == 0), stop=(k == KC - 1))
                nc.scalar.activation(pring[:, u:u + 1], pc, AF.Identity, bias=boutb)

            if groups > 1:
                with tc.For_i(0, groups, 1) as gi:
                    for u in range(unroll):
                        emit_step(u)
                    nc.sync.dma_start(preds_d[:, ds(gi * unroll, unroll)], pring)
            else:
                for u in range(unroll):
                    emit_step(u)
                nc.sync.dma_start(preds_d[:, 0:unroll], pring)

    nc.compile()
    return nc


def _host_inputs(inputs, T):
    """Host-side preprocessing: permute gate rows, cast, transpose, shard."""
    perm = _gate_perm()
    if MM_DTYPE == "f16":
        f16 = np.float16
    else:
        import ml_dtypes
        f16 = ml_dtypes.bfloat16
    f32 = np.float32

    def kchunks(W):  # [4H, H] -> [128, KC*4H], chunk k = W[perm][:, 128k:...].T
        Wp = W[perm]
        return np.ascontiguousarray(
            Wp.T.reshape(KC, 128, FOURH).transpose(1, 0, 2).reshape(128, KC * FOURH)
        )

    def gateblocks(W):  # [4H, H] -> [128, NCH*4H], block n cols [k*512+j]
        Wp = W[perm].reshape(NCH, 512, KC, 128)  # [n, j, k, p]
        return np.ascontiguousarray(
            Wp.transpose(3, 0, 2, 1).reshape(128, NCH * KC * 512)
        )

    w0 = kchunks(inputs["W_hh0"]).astype(f16)
    w1h = kchunks(inputs["W_hh1"]).astype(f16)
    w1i = gateblocks(inputs["W_ih1"]).astype(f16)
    wxb0 = np.zeros((32, FOURH), f16)
    wxb0[0] = inputs["W_ih0"][perm, 0].astype(f16)
    wxb0[1] = (inputs["b_ih0"] + inputs["b_hh0"])[perm].astype(f16)
    b1k = np.zeros((32, FOURH), f16)
    b1k[0] = (inputs["b_ih1"] + inputs["b_hh1"])[perm].astype(f16)
    xk0 = np.zeros((32, 128), f16)
    xk0[1] = 1.0
    ones32 = np.zeros((32, 128), f16)
    ones32[0] = 1.0
    wout = np.ascontiguousarray(inputs["W_out"][0].reshape(KC, 128).T).astype(f16)
    boutb = np.full((128, 1), inputs["b_out"][0], f32)
    bouts = np.full((1, 1), inputs["b_out"][0], f32)
    ident = np.eye(128, dtype=f16)

    def hT(h):  # [128b, H] -> [128p, H] with [p, 128k+b] = h[b, 128k+p]
        return np.ascontiguousarray(
            h.T.reshape(KC, 128, 128).transpose(1, 0, 2).reshape(128, H)
        )

    in_maps = []
    for c in range(N_CORES):
        bs = slice(c * B_LOC, (c + 1) * B_LOC)
        in_maps.append({
            "w0": w0, "w1h": w1h, "w1i": w1i, "wxb0": wxb0, "b1k": b1k,
            "xk0": xk0, "ones32": ones32,
            "hT0": hT(inputs["h0"][0][bs]).astype(f16),
            "hT1": hT(inputs["h0"][1][bs]).astype(f16),
            "c0s": inputs["c0"][0][bs].astype(f32),
            "c1s": inputs["c0"][1][bs].astype(f32),
            "woutT": wout, "boutb": boutb, "bouts": bouts,
            "ident": ident,
        })
    return in_maps


_NC_CACHE = {}


def _get_nc(T, unroll, passes=1):
    key = (T, unroll, passes)
    if key not in _NC_CACHE:
        _NC_CACHE[key] = _build_nc(T, unroll, passes)
    return _NC_CACHE[key]


def run_device(inputs, T=T_TOTAL, unroll=8, passes=1, **run_kwargs):
    """Run the Bass kernel on 8 cores; returns (preds [1024, T], results obj)."""
    nc = _get_nc(T, unroll, passes)
    in_maps = _host_inputs(inputs, T)
    res = run_bass_kernel_spmd(nc, in_maps, core_ids=list(range(N_CORES)),
                               **run_kwargs)
    preds = np.concatenate([r["preds"][:, :T] for r in res.results], axis=0)
    return preds, res


def kernel(**inputs):
    inputs = {k: np.asarray(v) for k, v in inputs.items()}
    B, T = inputs["outputs"].shape
    preds, _ = run_device(inputs, T=T)
    targets = inputs["outputs"].astype(np.float64)
    loss = np.float32(np.sum(np.mean((preds.astype(np.float64) - targets) ** 2,
                                     axis=0)))
    return preds.astype(np.float32), loss


# revision 51
# speedup vs baseline: 1.1390x; 1.1390x over previous
"""Trainium2 Bass kernel for nn_Decoder: 2-layer LSTM decoder with autoregressive
feedback (B=1024, T=256, H=1024), data-parallel over 8 NeuronCores.

Strategy:
  - Shard batch 1024 -> 128 per core; replicate weights (streamed/resident).
  - Per-core layout: batch on partitions, gate features on free dim.
  - fp16 matmul operands (weights + h), fp32 PSUM accumulation, fp32 cell state.
  - Gate rows pre-permuted on host into 8 chunks of [i|f|o|g] x 128 so one PSUM
    bank holds a complete gate quad for a 128-wide hidden slice.
  - bias + x-feedback folded into matmul via a K=2 stationary [ones; x].
  - h transposed each step on the PE (identity matmul) into fp16 h^T chunks.
  - W_hh0, W_hh1 resident in SBUF; W_ih1 streamed per step (gate-chunk blocks).
"""

import os

import numpy as np

import concourse.bass as bass
import concourse.mybir as mybir
import concourse.tile as tile
from concourse import bacc
from concourse.bass import ds
from concourse.bass_utils import run_bass_kernel_spmd

MM_DTYPE = os.environ.get("MM_DTYPE", "f16")
F16 = mybir.dt.float16 if MM_DTYPE == "f16" else mybir.dt.bfloat16
F32 = mybir.dt.float32
AF = mybir.ActivationFunctionType

DBG_NO_BIAS_MM = bool(int(os.environ.get("DBG_NO_BIAS_MM", "0")))
DBG_NO_TRANSPOSE = bool(int(os.environ.get("DBG_NO_TRANSPOSE", "0")))
DBG_NO_PRED = bool(int(os.environ.get("DBG_NO_PRED", "0")))
DBG_NO_STREAM = bool(int(os.environ.get("DBG_NO_STREAM", "0")))  # timing only
DBG_MM_ONLY = bool(int(os.environ.get("DBG_MM_ONLY", "0")))      # timing only

H = 1024
FOURH = 4 * H
KC = 8          # contraction chunks (H / 128)
NCH = 8         # gate chunks per layer (4096 / 512)
QUAD = int(os.environ.get("QUAD", "4"))  # chunks sharing one stationary load
B_LOC = 128
N_CORES = 8
T_TOTAL = 256


def _gate_perm():
    """Row permutation of the 4H gate dim: chunk n = [i_n | f_n | o_n | g_n],
    each 128 wide, covering hidden slice [128n, 128(n+1))."""
    idx = np.arange(128)
    parts = []
    for n in range(NCH):
        base = n * 128
        parts += [base + idx, H + base + idx, 3 * H + base + idx, 2 * H + base + idx]
    return np.concatenate(parts)


def _build_nc(T, unroll, passes=1):
    """Build the Bass module (single-core program, replicated SPMD on 8 cores).

    passes > 1 repeats the whole T-step loop (timing builds only: wall-clock
    difference between passes isolates pure device time)."""
    assert T % unroll == 0 and unroll % 2 == 0
    groups = (T // unroll) * passes

    nc = bacc.Bacc("TRN2", target_bir_lowering=False, debug=False)

    dI = lambda name, shape, dt: nc.dram_tensor(name, shape, dt, kind="ExternalInput")
    w0_d = dI("w0", [128, KC * FOURH], F16)       # W_hh0^T, k-chunk blocks
    w1h_d = dI("w1h", [128, KC * FOURH], F16)     # W_hh1^T, k-chunk blocks
    w1i_d = dI("w1i", [128, NCH * FOURH], F16)    # W_ih1^T, gate-chunk blocks
    # K=32-padded bias/x operands (K<32 matmuls hang TRN2):
    #   wxb0: row0 = W_ih0 (perm), row1 = b_ih0+b_hh0 (perm), rows 2-31 = 0
    #   b1k:  row0 = b_ih1+b_hh1 (perm), rows 1-31 = 0
    wxb0_d = dI("wxb0", [32, FOURH], F16)
    b1k_d = dI("b1k", [32, FOURH], F16)
    # xk32: row0 = x (runtime), row1 = ones, rest 0; ones32: row0 = ones
    xk0_d = dI("xk0", [32, 128], F16)
    ones32_d = dI("ones32", [32, 128], F16)
    hT0_d = dI("hT0", [128, H], F16)
    hT1_d = dI("hT1", [128, H], F16)
    c0_d = dI("c0s", [128, H], F32)
    c1_d = dI("c1s", [128, H], F32)
    wout_d = dI("woutT", [128, KC], F16)
    boutb_d = dI("boutb", [128, 1], F32)
    bouts_d = dI("bouts", [1, 1], F32)
    ident_d = dI("ident", [128, 128], F16)
    preds_d = nc.dram_tensor("preds", [128, T * passes], F32,
                             kind="ExternalOutput")

    with tile.TileContext(nc) as tc:
        with (
            tc.tile_pool(name="state", bufs=1) as state,
            tc.tile_pool(name="wstream", bufs=2) as wstream,
            tc.tile_pool(name="work", bufs=3) as work,
            tc.tile_pool(name="gpsum", bufs=QUAD, space="PSUM") as gpsum,
            tc.tile_pool(name="tpsum", bufs=2, space="PSUM") as tpsum,
            tc.tile_pool(name="ppsum", bufs=1, space="PSUM") as ppsum,
        ):
            # ---- persistent SBUF state -------------------------------------
            w0_sb = state.tile([128, KC * FOURH], F16, name="w0_sb")
            w1h_sb = state.tile([128, KC * FOURH], F16, name="w1h_sb")
            wxb0_sb = state.tile([32, FOURH], F16, name="wxb0_sb")
            b1k_sb = state.tile([32, FOURH], F16, name="b1k_sb")
            hT0_pp = [state.tile([128, H], F16, name=f"hT0_{p}") for p in range(2)]
            hT1_pp = [state.tile([128, H], F16, name=f"hT1_{p}") for p in range(2)]
            h0st = state.tile([128, H], F16, name="h0st")
            h1st = state.tile([128, H], F16, name="h1st")
            c0_sb = state.tile([128, H], F32, name="c0_sb")
            c1_sb = state.tile([128, H], F32, name="c1_sb")
            xk32 = state.tile([32, 128], F16, name="xk32")
            ones32 = state.tile([32, 128], F16, name="ones32")
            ident = state.tile([128, 128], F16, name="ident")
            woutT = state.tile([128, KC], F16, name="woutT")
            boutb = state.tile([128, 1], F32, name="boutb")
            bouts = state.tile([1, 1], F32, name="bouts")
            pring = state.tile([128, unroll], F32, name="pring")

            # ---- setup DMAs ------------------------------------------------
            nc.sync.dma_start(w0_sb, w0_d[:, :])
            nc.sync.dma_start(w1h_sb, w1h_d[:, :])
            nc.sync.dma_start(wxb0_sb, wxb0_d[:, :])
            nc.sync.dma_start(b1k_sb, b1k_d[:, :])
            nc.sync.dma_start(hT0_pp[0], hT0_d[:, :])
            nc.sync.dma_start(hT1_pp[0], hT1_d[:, :])
            nc.sync.dma_start(c0_sb, c0_d[:, :])
            nc.sync.dma_start(c1_sb, c1_d[:, :])
            nc.sync.dma_start(xk32, xk0_d[:, :])
            nc.sync.dma_start(ones32, ones32_d[:, :])
            nc.sync.dma_start(ident, ident_d[:, :])

            if DBG_NO_STREAM:
                w1i_fake = state.tile([128, FOURH], F16, name="w1i_fake")
                nc.sync.dma_start(w1i_fake, w1i_d[:, 0:FOURH])
            nc.sync.dma_start(woutT, wout_d[:, :])
            nc.sync.dma_start(boutb, boutb_d[:, :])
            nc.sync.dma_start(bouts, bouts_d[:, :])

            def process_chunk(n, ps, c_sb, hst, hT_new):
                """Gate-chunk n post-matmul: activations, cell update, h write
                (fp16) and PE transpose into hT_new."""
                if DBG_MM_ONLY:
                    return
                nsl = slice(n * 128, (n + 1) * 128)
                sif = work.tile([128, 384], F32, tag="sif", name="sif")
                nc.scalar.activation(sif, ps[:, 0:384], AF.Sigmoid)
                tg = work.tile([128, 128], F32, tag="tg", name="tg")
                nc.scalar.activation(tg, ps[:, 384:512], AF.Tanh)
                prod = work.tile([128, 128], F32, tag="prod", name="prod")
                nc.vector.tensor_mul(prod, sif[:, 0:128], tg)
                tmp = work.tile([128, 128], F32, tag="tmp", name="tmp")
                nc.vector.tensor_mul(tmp, sif[:, 128:256], c_sb[:, nsl])
                nc.vector.tensor_add(c_sb[:, nsl], tmp, prod)
                tct = work.tile([128, 128], F32, tag="tct", name="tct")
                nc.scalar.activation(tct, c_sb[:, nsl], AF.Tanh)
                nc.vector.tensor_mul(hst[:, nsl], sif[:, 256:384], tct)
                if DBG_NO_TRANSPOSE:
                    nc.vector.tensor_copy(hT_new[:, nsl], hst[:, nsl])
                else:
                    pt = tpsum.tile([128, 128], F16, tag="tp", name="pt")
                    nc.tensor.transpose(pt, hst[:, nsl], ident)
                    nc.vector.tensor_copy(hT_new[:, nsl], pt)

            def emit_step(u):
                hT0_cur, hT0_new = hT0_pp[u % 2], hT0_pp[(u + 1) % 2]
                hT1_cur, hT1_new = hT1_pp[u % 2], hT1_pp[(u + 1) % 2]
                if DBG_MM_ONLY:
                    hT0_cur = hT0_new = hT0_pp[0]
                    hT1_cur = hT1_new = hT1_pp[0]

                # stream W_ih1 gate-chunk blocks for this step
                if DBG_NO_STREAM:
                    st = [w1i_fake] * NCH
                else:
                    st = []
                    for n in range(NCH):
                        s = wstream.tile([128, FOURH], F16, tag="w1is", name="w1is")
                        nc.sync.dma_start(s, w1i_d[:, n * FOURH:(n + 1) * FOURH])
                        st.append(s)

                # ---- layer 0 ----  (k-outer over chunk-quads: the stationary
                # hT chunk is shared by 4 consecutive matmuls)
                for q in range(NCH // QUAD):
                    ns = range(QUAD * q, QUAD * q + QUAD)
                    pss = {n: gpsum.tile([128, 512], F32, tag="g", name="ps")
                           for n in ns}
                    for k in range(KC):
                        for n in ns:
                            nc.tensor.matmul(
                                pss[n],
                                hT0_cur[:, k * 128:(k + 1) * 128],
                                w0_sb[:, k * FOURH + n * 512: k * FOURH + (n + 1) * 512],
                                start=(k == 0), stop=DBG_NO_BIAS_MM and k == KC - 1,
                            )
                    if not DBG_NO_BIAS_MM:
                        for n in ns:
                            nc.tensor.matmul(
                                pss[n], xk32[:, :],
                                wxb0_sb[:, n * 512:(n + 1) * 512],
                                start=False, stop=True)
                    for n in ns:
                        process_chunk(n, pss[n], c0_sb, h0st, hT0_new)

                # ---- layer 1 ----
                for q in range(NCH // QUAD):
                    ns = range(QUAD * q, QUAD * q + QUAD)
                    pss = {n: gpsum.tile([128, 512], F32, tag="g", name="ps")
                           for n in ns}
                    for k in range(KC):
                        for n in ns:
                            nc.tensor.matmul(
                                pss[n],
                                hT1_cur[:, k * 128:(k + 1) * 128],
                                w1h_sb[:, k * FOURH + n * 512: k * FOURH + (n + 1) * 512],
                                start=(k == 0), stop=False,
                            )
                    for k in range(KC):
                        for n in ns:
                            nc.tensor.matmul(
                                pss[n],
                                hT0_new[:, k * 128:(k + 1) * 128],
                                st[n][:, k * 512:(k + 1) * 512],
                                start=False,
                                stop=DBG_NO_BIAS_MM and k == KC - 1,
                            )
                    if not DBG_NO_BIAS_MM:
                        for n in ns:
                            nc.tensor.matmul(
                                pss[n], ones32[:, :],
                                b1k_sb[:, n * 512:(n + 1) * 512],
                                start=False, stop=True)
                    for n in ns:
                        process_chunk(n, pss[n], c1_sb, h1st, hT1_new)

                # ---- prediction ----
                if DBG_NO_PRED:
                    nc.scalar.activation(pring[:, u:u + 1], c1_sb[:, 0:1],
                                         AF.Identity, bias=boutb)
                    return
                # row form [1, 128] -> next-step x (fp16, into xones row 1)
                pr = ppsum.tile([1, 128], F32, tag="predr", name="pr")
                for k in range(KC):
                    nc.tensor.matmul(pr, woutT[:, k:k + 1],
                                     hT1_new[:, k * 128:(k + 1) * 128],
                                     start=(k == 0), stop=(k == KC - 1))
                nc.scalar.activation(xk32[0:1, :], pr, AF.Identity, bias=bouts)
                # column form [128, 1] -> preds ring (fp32)
                pc = ppsum.tile([128, 1], F32, tag="predc", name="pc")
                for k in range(KC):
                    nc.tensor.matmul(pc, hT1_new[:, k * 128:(k + 1) * 128],
                                     woutT[:, k:k + 1],
                                     start=(k == 0), stop=(k == KC - 1))
                nc.scalar.activation(pring[:, u:u + 1], pc, AF.Identity, bias=boutb)

            if groups > 1:
                with tc.For_i(0, groups, 1) as gi:
                    for u in range(unroll):
                        emit_step(u)
                    nc.sync.dma_start(preds_d[:, ds(gi * unroll, unroll)], pring)
            else:
                for u in range(unroll):
                    emit_step(u)
                nc.sync.dma_start(preds_d[:, 0:unroll], pring)

    nc.compile()
    return nc


def _host_inputs(inputs, T):
    """Host-side preprocessing: permute gate rows, cast, transpose, shard."""
    perm = _gate_perm()
    if MM_DTYPE == "f16":
        f16 = np.float16
    else:
        import ml_dtypes
        f16 = ml_dtypes.bfloat16
    f32 = np.float32

    def kchunks(W):  # [4H, H] -> [128, KC*4H], chunk k = W[perm][:, 128k:...].T
        Wp = W[perm]
        return np.ascontiguousarray(
            Wp.T.reshape(KC, 128, FOURH).transpose(1, 0, 2).reshape(128, KC * FOURH)
        )

    def gateblocks(W):  # [4H, H] -> [128, NCH*4H], block n cols [k*512+j]
        Wp = W[perm].reshape(NCH, 512, KC, 128)  # [n, j, k, p]
        return np.ascontiguousarray(
            Wp.transpose(3, 0, 2, 1).reshape(128, NCH * KC * 512)
        )

    w0 = kchunks(inputs["W_hh0"]).astype(f16)
    w1h = kchunks(inputs["W_hh1"]).astype(f16)
    w1i = gateblocks(inputs["W_ih1"]).astype(f16)
    wxb0 = np.zeros((32, FOURH), f16)
    wxb0[0] = inputs["W_ih0"][perm, 0].astype(f16)
    wxb0[1] = (inputs["b_ih0"] + inputs["b_hh0"])[perm].astype(f16)
    b1k = np.zeros((32, FOURH), f16)
    b1k[0] = (inputs["b_ih1"] + inputs["b_hh1"])[perm].astype(f16)
    xk0 = np.zeros((32, 128), f16)
    xk0[1] = 1.0
    ones32 = np.zeros((32, 128), f16)
    ones32[0] = 1.0
    wout = np.ascontiguousarray(inputs["W_out"][0].reshape(KC, 128).T).astype(f16)
    boutb = np.full((128, 1), inputs["b_out"][0], f32)
    bouts = np.full((1, 1), inputs["b_out"][0], f32)
    ident = np.eye(128, dtype=f16)

    def hT(h):  # [128b, H] -> [128p, H] with [p, 128k+b] = h[b, 128k+p]
        return np.ascontiguousarray(
            h.T.reshape(KC, 128, 128).transpose(1, 0, 2).reshape(128, H)
        )

    in_maps = []
    for c in range(N_CORES):
        bs = slice(c * B_LOC, (c + 1) * B_LOC)
        in_maps.append({
            "w0": w0, "w1h": w1h, "w1i": w1i, "wxb0": wxb0, "b1k": b1k,
            "xk0": xk0, "ones32": ones32,
            "hT0": hT(inputs["h0"][0][bs]).astype(f16),
            "hT1": hT(inputs["h0"][1][bs]).astype(f16),
            "c0s": inputs["c0"][0][bs].astype(f32),
            "c1s": inputs["c0"][1][bs].astype(f32),
            "woutT": wout, "boutb": boutb, "bouts": bouts,
            "ident": ident,
        })
    return in_maps


_NC_CACHE = {}


def _get_nc(T, unroll, passes=1):
    key = (T, unroll, passes)
    if key not in _NC_CACHE:
        _NC_CACHE[key] = _build_nc(T, unroll, passes)
    return _NC_CACHE[key]


def run_device(inputs, T=T_TOTAL, unroll=8, passes=1, **run_kwargs):
    """Run the Bass kernel on 8 cores; returns (preds [1024, T], results obj)."""
    nc = _get_nc(T, unroll, passes)
    in_maps = _host_inputs(inputs, T)
    res = run_bass_kernel_spmd(nc, in_maps, core_ids=list(range(N_CORES)),
                               **run_kwargs)
    preds = np.concatenate([r["preds"][:, :T] for r in res.results], axis=0)
    return preds, res


def kernel(**inputs):
    inputs = {k: np.asarray(v) for k, v in inputs.items()}
    B, T = inputs["outputs"].shape
    preds, _ = run_device(inputs, T=T)
    targets = inputs["outputs"].astype(np.float64)
    loss = np.float32(np.sum(np.mean((preds.astype(np.float64) - targets) ** 2,
                                     axis=0)))
    return preds.astype(np.float32), loss


# revision 55
# speedup vs baseline: 1.4586x; 1.2805x over previous
"""Trainium2 Bass kernel for nn_Decoder: 2-layer LSTM decoder with autoregressive
feedback (B=1024, T=256, H=1024), data-parallel over 8 NeuronCores.

Strategy:
  - Shard batch 1024 -> 128 per core; replicate weights (streamed/resident).
  - Per-core layout: batch on partitions, gate features on free dim.
  - fp16 matmul operands (weights + h), fp32 PSUM accumulation, fp32 cell state.
  - Gate rows pre-permuted on host into 8 chunks of [i|f|o|g] x 128 so one PSUM
    bank holds a complete gate quad for a 128-wide hidden slice.
  - bias + x-feedback folded into matmul via a K=2 stationary [ones; x].
  - h transposed each step on the PE (identity matmul) into fp16 h^T chunks.
  - W_hh0, W_hh1 resident in SBUF; W_ih1 streamed per step (gate-chunk blocks).
"""

import os

import numpy as np

import concourse.bass as bass
import concourse.mybir as mybir
import concourse.tile as tile
from concourse import bacc
from concourse.bass import ds
from concourse.bass_utils import run_bass_kernel_spmd

MM_DTYPE = os.environ.get("MM_DTYPE", "f16")
F16 = mybir.dt.float16 if MM_DTYPE == "f16" else mybir.dt.bfloat16
F32 = mybir.dt.float32
AF = mybir.ActivationFunctionType

DBG_NO_BIAS_MM = bool(int(os.environ.get("DBG_NO_BIAS_MM", "0")))
DBG_NO_TRANSPOSE = bool(int(os.environ.get("DBG_NO_TRANSPOSE", "0")))
DBG_NO_PRED = bool(int(os.environ.get("DBG_NO_PRED", "0")))
DBG_NO_STREAM = bool(int(os.environ.get("DBG_NO_STREAM", "0")))  # timing only
DBG_MM_ONLY = bool(int(os.environ.get("DBG_MM_ONLY", "0")))      # timing only
DBG_NSPLIT = int(os.environ.get("DBG_NSPLIT", "1"))              # timing only
DBG_KSTEP = int(os.environ.get("DBG_KSTEP", "1"))                # timing only

H = 1024
FOURH = 4 * H
KC = 8          # contraction chunks (H / 128)
NCH = 8         # gate chunks per layer (4096 / 512)
QUAD = int(os.environ.get("QUAD", "4"))  # chunks sharing one stationary load
B_LOC = 128
N_CORES = 8
T_TOTAL = 256


def _gate_perm():
    """Row permutation of the 4H gate dim: chunk n = [i_n | f_n | o_n | g_n],
    each 128 wide, covering hidden slice [128n, 128(n+1))."""
    idx = np.arange(128)
    parts = []
    for n in range(NCH):
        base = n * 128
        parts += [base + idx, H + base + idx, 3 * H + base + idx, 2 * H + base + idx]
    return np.concatenate(parts)


def _build_nc(T, unroll, passes=1):
    """Build the Bass module (single-core program, replicated SPMD on 8 cores).

    passes > 1 repeats the whole T-step loop (timing builds only: wall-clock
    difference between passes isolates pure device time)."""
    assert T % unroll == 0 and unroll % 2 == 0
    groups = (T // unroll) * passes

    nc = bacc.Bacc("TRN2", target_bir_lowering=False, debug=False)

    dI = lambda name, shape, dt: nc.dram_tensor(name, shape, dt, kind="ExternalInput")
    w0_d = dI("w0", [128, KC * FOURH], F16)       # W_hh0^T, k-chunk blocks
    w1h_d = dI("w1h", [128, KC * FOURH], F16)     # W_hh1^T, k-chunk blocks
    w1i_d = dI("w1i", [128, NCH * FOURH], F16)    # W_ih1^T, gate-chunk blocks
    # K=32-padded bias/x operands (K<32 matmuls hang TRN2):
    #   wxb0: row0 = W_ih0 (perm), row1 = b_ih0+b_hh0 (perm), rows 2-31 = 0
    #   b1k:  row0 = b_ih1+b_hh1 (perm), rows 1-31 = 0
    wxb0_d = dI("wxb0", [32, FOURH], F16)
    b1k_d = dI("b1k", [32, FOURH], F16)
    # xk32: row0 = x (runtime), row1 = ones, rest 0; ones32: row0 = ones
    xk0_d = dI("xk0", [32, 128], F16)
    ones32_d = dI("ones32", [32, 128], F16)
    hT0_d = dI("hT0", [128, H], F16)
    hT1_d = dI("hT1", [128, H], F16)
    c0_d = dI("c0s", [128, H], F32)
    c1_d = dI("c1s", [128, H], F32)
    wout_d = dI("woutT", [128, KC], F16)
    boutb_d = dI("boutb", [128, 1], F32)
    bouts_d = dI("bouts", [1, 1], F32)
    ident_d = dI("ident", [128, 128], F16)
    preds_d = nc.dram_tensor("preds", [128, T * passes], F32,
                             kind="ExternalOutput")

    with tile.TileContext(nc) as tc:
        with (
            tc.tile_pool(name="state", bufs=1) as state,
            tc.tile_pool(name="wstream", bufs=2) as wstream,
            tc.tile_pool(name="work", bufs=3) as work,
            tc.tile_pool(name="gpsum", bufs=QUAD, space="PSUM") as gpsum,
            tc.tile_pool(name="tpsum", bufs=2, space="PSUM") as tpsum,
            tc.tile_pool(name="ppsum", bufs=1, space="PSUM") as ppsum,
        ):
            # ---- persistent SBUF state -------------------------------------
            w0_sb = state.tile([128, KC * FOURH], F16, name="w0_sb")
            w1h_sb = state.tile([128, KC * FOURH], F16, name="w1h_sb")
            wxb0_sb = state.tile([32, FOURH], F16, name="wxb0_sb")
            b1k_sb = state.tile([32, FOURH], F16, name="b1k_sb")
            hT0_pp = [state.tile([128, H], F16, name=f"hT0_{p}") for p in range(2)]
            hT1_pp = [state.tile([128, H], F16, name=f"hT1_{p}") for p in range(2)]
            h0st = state.tile([128, H], F16, name="h0st")
            h1st = state.tile([128, H], F16, name="h1st")
            c0_sb = state.tile([128, H], F32, name="c0_sb")
            c1_sb = state.tile([128, H], F32, name="c1_sb")
            xk32 = state.tile([32, 128], F16, name="xk32")
            ones32 = state.tile([32, 128], F16, name="ones32")
            ident = state.tile([128, 128], F16, name="ident")
            woutT = state.tile([128, KC], F16, name="woutT")
            boutb = state.tile([128, 1], F32, name="boutb")
            bouts = state.tile([1, 1], F32, name="bouts")
            pring = state.tile([128, unroll], F32, name="pring")

            # ---- setup DMAs ------------------------------------------------
            nc.sync.dma_start(w0_sb, w0_d[:, :])
            nc.sync.dma_start(w1h_sb, w1h_d[:, :])
            nc.sync.dma_start(wxb0_sb, wxb0_d[:, :])
            nc.sync.dma_start(b1k_sb, b1k_d[:, :])
            nc.sync.dma_start(hT0_pp[0], hT0_d[:, :])
            nc.sync.dma_start(hT1_pp[0], hT1_d[:, :])
            nc.sync.dma_start(c0_sb, c0_d[:, :])
            nc.sync.dma_start(c1_sb, c1_d[:, :])
            nc.sync.dma_start(xk32, xk0_d[:, :])
            nc.sync.dma_start(ones32, ones32_d[:, :])
            nc.sync.dma_start(ident, ident_d[:, :])

            if DBG_NO_STREAM:
                w1i_fake = state.tile([128, FOURH], F16, name="w1i_fake")
                nc.sync.dma_start(w1i_fake, w1i_d[:, 0:FOURH])
            nc.sync.dma_start(woutT, wout_d[:, :])
            nc.sync.dma_start(boutb, boutb_d[:, :])
            nc.sync.dma_start(bouts, bouts_d[:, :])

            def w_mm(ps, lhsT, rhs, start, stop):
                """One [128,512] W matmul, optionally split along N (debug)."""
                w = 512 // DBG_NSPLIT
                for i in range(DBG_NSPLIT):
                    nc.tensor.matmul(ps[:, i * w:(i + 1) * w], lhsT,
                                     rhs[:, i * w:(i + 1) * w],
                                     start=start, stop=stop)

            def process_chunk(n, ps, c_sb, hst, hT_new):
                """Gate-chunk n post-matmul: activations, cell update, h write
                (fp16) and PE transpose into hT_new."""
                if DBG_MM_ONLY:
                    return
                nsl = slice(n * 128, (n + 1) * 128)
                sif = work.tile([128, 384], F32, tag="sif", name="sif")
                nc.scalar.activation(sif, ps[:, 0:384], AF.Sigmoid)
                tg = work.tile([128, 128], F32, tag="tg", name="tg")
                nc.scalar.activation(tg, ps[:, 384:512], AF.Tanh)
                prod = work.tile([128, 128], F32, tag="prod", name="prod")
                nc.vector.tensor_mul(prod, sif[:, 0:128], tg)
                tmp = work.tile([128, 128], F32, tag="tmp", name="tmp")
                nc.vector.tensor_mul(tmp, sif[:, 128:256], c_sb[:, nsl])
                nc.vector.tensor_add(c_sb[:, nsl], tmp, prod)
                tct = work.tile([128, 128], F32, tag="tct", name="tct")
                nc.scalar.activation(tct, c_sb[:, nsl], AF.Tanh)
                nc.vector.tensor_mul(hst[:, nsl], sif[:, 256:384], tct)
                if DBG_NO_TRANSPOSE:
                    nc.vector.tensor_copy(hT_new[:, nsl], hst[:, nsl])
                else:
                    pt = tpsum.tile([128, 128], F16, tag="tp", name="pt")
                    nc.tensor.transpose(pt, hst[:, nsl], ident)
                    nc.vector.tensor_copy(hT_new[:, nsl], pt)

            def emit_step(u):
                hT0_cur, hT0_new = hT0_pp[u % 2], hT0_pp[(u + 1) % 2]
                hT1_cur, hT1_new = hT1_pp[u % 2], hT1_pp[(u + 1) % 2]
                if DBG_MM_ONLY:
                    hT0_cur = hT0_new = hT0_pp[0]
                    hT1_cur = hT1_new = hT1_pp[0]

                # stream W_ih1 gate-chunk blocks for this step
                if DBG_NO_STREAM:
                    st = [w1i_fake] * NCH
                else:
                    st = []
                    for n in range(NCH):
                        s = wstream.tile([128, FOURH], F16, tag="w1is", name="w1is")
                        nc.sync.dma_start(s, w1i_d[:, n * FOURH:(n + 1) * FOURH])
                        st.append(s)

                # ---- layer 0 ----  (k-outer over chunk-quads: the stationary
                # hT chunk is shared by 4 consecutive matmuls)
                for q in range(NCH // QUAD):
                    ns = range(QUAD * q, QUAD * q + QUAD)
                    pss = {n: gpsum.tile([128, 512], F32, tag="g", name="ps")
                           for n in ns}
                    for k in range(0, KC, DBG_KSTEP):
                        for n in ns:
                            w_mm(
                                pss[n],
                                hT0_cur[:, k * 128:(k + 1) * 128],
                                w0_sb[:, k * FOURH + n * 512: k * FOURH + (n + 1) * 512],
                                start=(k == 0), stop=DBG_NO_BIAS_MM and k >= KC - DBG_KSTEP,
                            )
                    if not DBG_NO_BIAS_MM:
                        for n in ns:
                            nc.tensor.matmul(
                                pss[n], xk32[:, :],
                                wxb0_sb[:, n * 512:(n + 1) * 512],
                                start=False, stop=True)
                    for n in ns:
                        process_chunk(n, pss[n], c0_sb, h0st, hT0_new)

                # ---- layer 1 ----
                for q in range(NCH // QUAD):
                    ns = range(QUAD * q, QUAD * q + QUAD)
                    pss = {n: gpsum.tile([128, 512], F32, tag="g", name="ps")
                           for n in ns}
                    for k in range(0, KC, DBG_KSTEP):
                        for n in ns:
                            w_mm(
                                pss[n],
                                hT1_cur[:, k * 128:(k + 1) * 128],
                                w1h_sb[:, k * FOURH + n * 512: k * FOURH + (n + 1) * 512],
                                start=(k == 0), stop=False,
                            )
                    for k in range(0, KC, DBG_KSTEP):
                        for n in ns:
                            w_mm(
                                pss[n],
                                hT0_new[:, k * 128:(k + 1) * 128],
                                st[n][:, k * 512:(k + 1) * 512],
                                start=False,
                                stop=DBG_NO_BIAS_MM and k >= KC - DBG_KSTEP,
                            )
                    if not DBG_NO_BIAS_MM:
                        for n in ns:
                            nc.tensor.matmul(
                                pss[n], ones32[:, :],
                                b1k_sb[:, n * 512:(n + 1) * 512],
                                start=False, stop=True)
                    for n in ns:
                        process_chunk(n, pss[n], c1_sb, h1st, hT1_new)

                # ---- prediction ----
                if DBG_NO_PRED:
                    nc.scalar.activation(pring[:, u:u + 1], c1_sb[:, 0:1],
                                         AF.Identity, bias=boutb)
                    return
                # row form [1, 128] -> next-step x (fp16, into xones row 1)
                pr = ppsum.tile([1, 128], F32, tag="predr", name="pr")
                for k in range(KC):
                    nc.tensor.matmul(pr, woutT[:, k:k + 1],
                                     hT1_new[:, k * 128:(k + 1) * 128],
                                     start=(k == 0), stop=(k == KC - 1))
                nc.scalar.activation(xk32[0:1, :], pr, AF.Identity, bias=bouts)
                # column form [128, 1] -> preds ring (fp32)
                pc = ppsum.tile([128, 1], F32, tag="predc", name="pc")
                for k in range(KC):
                    nc.tensor.matmul(pc, hT1_new[:, k * 128:(k + 1) * 128],
                                     woutT[:, k:k + 1],
                                     start=(k == 0), stop=(k == KC - 1))
                nc.scalar.activation(pring[:, u:u + 1], pc, AF.Identity, bias=boutb)

            if groups > 1:
                with tc.For_i(0, groups, 1) as gi:
                    for u in range(unroll):
                        emit_step(u)
                    nc.sync.dma_start(preds_d[:, ds(gi * unroll, unroll)], pring)
            else:
                for u in range(unroll):
                    emit_step(u)
                nc.sync.dma_start(preds_d[:, 0:unroll], pring)

    nc.compile()
    return nc


def _host_inputs(inputs, T):
    """Host-side preprocessing: permute gate rows, cast, transpose, shard."""
    perm = _gate_perm()
    if MM_DTYPE == "f16":
        f16 = np.float16
    else:
        import ml_dtypes
        f16 = ml_dtypes.bfloat16
    f32 = np.float32

    def kchunks(W):  # [4H, H] -> [128, KC*4H], chunk k = W[perm][:, 128k:...].T
        Wp = W[perm]
        return np.ascontiguousarray(
            Wp.T.reshape(KC, 128, FOURH).transpose(1, 0, 2).reshape(128, KC * FOURH)
        )

    def gateblocks(W):  # [4H, H] -> [128, NCH*4H], block n cols [k*512+j]
        Wp = W[perm].reshape(NCH, 512, KC, 128)  # [n, j, k, p]
        return np.ascontiguousarray(
            Wp.transpose(3, 0, 2, 1).reshape(128, NCH * KC * 512)
        )

    w0 = kchunks(inputs["W_hh0"]).astype(f16)
    w1h = kchunks(inputs["W_hh1"]).astype(f16)
    w1i = gateblocks(inputs["W_ih1"]).astype(f16)
    wxb0 = np.zeros((32, FOURH), f16)
    wxb0[0] = inputs["W_ih0"][perm, 0].astype(f16)
    wxb0[1] = (inputs["b_ih0"] + inputs["b_hh0"])[perm].astype(f16)
    b1k = np.zeros((32, FOURH), f16)
    b1k[0] = (inputs["b_ih1"] + inputs["b_hh1"])[perm].astype(f16)
    xk0 = np.zeros((32, 128), f16)
    xk0[1] = 1.0
    ones32 = np.zeros((32, 128), f16)
    ones32[0] = 1.0
    wout = np.ascontiguousarray(inputs["W_out"][0].reshape(KC, 128).T).astype(f16)
    boutb = np.full((128, 1), inputs["b_out"][0], f32)
    bouts = np.full((1, 1), inputs["b_out"][0], f32)
    ident = np.eye(128, dtype=f16)

    def hT(h):  # [128b, H] -> [128p, H] with [p, 128k+b] = h[b, 128k+p]
        return np.ascontiguousarray(
            h.T.reshape(KC, 128, 128).transpose(1, 0, 2).reshape(128, H)
        )

    in_maps = []
    for c in range(N_CORES):
        bs = slice(c * B_LOC, (c + 1) * B_LOC)
        in_maps.append({
            "w0": w0, "w1h": w1h, "w1i": w1i, "wxb0": wxb0, "b1k": b1k,
            "xk0": xk0, "ones32": ones32,
            "hT0": hT(inputs["h0"][0][bs]).astype(f16),
            "hT1": hT(inputs["h0"][1][bs]).astype(f16),
            "c0s": inputs["c0"][0][bs].astype(f32),
            "c1s": inputs["c0"][1][bs].astype(f32),
            "woutT": wout, "boutb": boutb, "bouts": bouts,
            "ident": ident,
        })
    return in_maps


_NC_CACHE = {}


def _get_nc(T, unroll, passes=1):
    key = (T, unroll, passes)
    if key not in _NC_CACHE:
        _NC_CACHE[key] = _build_nc(T, unroll, passes)
    return _NC_CACHE[key]


def run_device(inputs, T=T_TOTAL, unroll=8, passes=1, **run_kwargs):
    """Run the Bass kernel on 8 cores; returns (preds [1024, T], results obj)."""
    nc = _get_nc(T, unroll, passes)
    in_maps = _host_inputs(inputs, T)
    res = run_bass_kernel_spmd(nc, in_maps, core_ids=list(range(N_CORES)),
                               **run_kwargs)
    preds = np.concatenate([r["preds"][:, :T] for r in res.results], axis=0)
    return preds, res


def kernel(**inputs):
    inputs = {k: np.asarray(v) for k, v in inputs.items()}
    B, T = inputs["outputs"].shape
    preds, _ = run_device(inputs, T=T)
    targets = inputs["outputs"].astype(np.float64)
    loss = np.float32(np.sum(np.mean((preds.astype(np.float64) - targets) ** 2,
                                     axis=0)))
    return preds.astype(np.float32), loss
